# revision 1
# baseline (speedup 1.0000x reference)
"""DUMA kernel: full inputs -> full output, 8 NeuronCores data-parallel over batch.

Development version: imports sibling modules. Will be inlined for submission.
"""
import numpy as np

_state = {}


def _ensure_built():
    if "runner" in _state:
        return
    import concourse.bacc as bacc
    import duma_builder as db
    from bass_runner import BassRunner

    nc = bacc.Bacc("TRN2", target_bir_lowering=False, debug=False,
                   num_devices=8)
    t = db.declare_io(nc)
    db.build(nc, t)
    nc.compile()
    _state["nc"] = nc
    _state["runner"] = BassRunner(nc, 8)


def prep_in_maps(inputs):
    import duma_host as dh
    return dh.prep_cores(inputs)


def get_runner():
    _ensure_built()
    return _state["runner"]


def kernel(**inputs) -> np.ndarray:
    _ensure_built()
    in_maps = prep_in_maps(inputs)
    res = _state["runner"].run(in_maps)
    out = np.stack([r["outT"].T for r in res])  # [B, S, H]
    return np.ascontiguousarray(out, dtype=np.float32)


def timeline_estimate() -> float:
    _ensure_built()
    if "tl" not in _state:
        from concourse.timeline_sim import TimelineSim
        tl = TimelineSim(_state["nc"], trace=False)
        _state["tl"] = tl.simulate()
    return _state["tl"]
